# revision 31
# baseline (speedup 1.0000x reference)
"""MinRNN Trainium2 kernel — parallel-in-time Picard iteration with exact
per-sweep diagonal solves on the DVE hardware scan.

Model (per batch row b):
    z_t = tanh(x_t @ W_in^T + b_in)                      # no recurrence
    u_t = sigmoid(s_{t-1} @ W_rec^T + z_t @ U_z^T + b_u)
    s_t = u_t * s_{t-1} + (1 - u_t) * z_t

Reformulate with m_t := s_t - z_t  (so s_t = z_t + m_t):
    pre_t = atil_t + W_rec m_{t-1},   atil_t = W_rec z_{t-1} + U_z z_t + b_u
    m_t   = sigmoid(pre_t) * (zd_t + m_{t-1}),  zd_t = z_{t-1} - z_t
(z_{-1} = 0, m_{-1} = 0 gives s_{-1} = 0.)

A sequential scan is LDWEIGHTS-bound on the PE (~850ns/step streaming all of
W_rec against a 2-wide operand => ~1.75ms floor).  Instead iterate on the
whole trajectory: per sweep,
    u^{(j)}_t = sigmoid(atil_t + W_rec m^{(j-1)}_{t-1})      # dense GEMM, 512-wide tiles
    m^{(j)}_t = (zd_t + m^{(j)}_{t-1}) * u^{(j)}_t           # EXACT diagonal solve:
                                                             # DVE tensor_tensor_scan
The exact diagonal solve makes the outer iteration contract ~5x per sweep on
this data (vs ~1.5x for plain Jacobi): 5 GEMM sweeps reach ~2.3e-3 max-rel
(gate is 2e-2).  atil is injected into PSUM with identity matmuls so the only
DVE work per sweep is the scan itself; scans run in two T/2 halves so they
overlap the next sweep's GEMM (its tiles 0-3 need only half 1, since tile i
reads m over t in [256*i-1, 256*(i+1)-2]).  All sweep tensors are fp16 (bf16
noise plateaus at the 2e-2 gate; fp16 at ~2e-3; scan state is fp32 in hw).

The kernel outputs z (fp16, written during phase A) and the final m (fp16);
the host computes s = z + m in fp32 — this removes a serialized on-device
output pass and one fp16 rounding.

Sharding: data-parallel over batch, 2 rows per core, no collectives.
"""

import numpy as np

import concourse.bass as bass
import concourse.mybir as mybir
import concourse.tile as tile
import concourse.bacc as bacc
from concourse import bass_utils
import concourse.alu_op_type as aot

AF = mybir.ActivationFunctionType
ET = mybir.EngineType

B, T, I, H = 16, 2048, 512, 512
N_CORES = 8
BL = B // N_CORES          # batch rows per core (2)
KC = I // 128              # input-dim chunks (4)
HC = H // 128              # hidden-dim chunks (4)
SWEEPS = 4                 # total sweeps incl. the GEMM-free first one

f32 = mybir.dt.float32
f32r = mybir.dt.float32r
f16 = mybir.dt.float16


def build(t_steps: int = T, tb: int = 0, sweeps: int = SWEEPS, compile: bool = True):
    """Build the per-core Bass program (same program on all 8 cores).
    `tb` accepted for CLI compat; tile width is fixed at min(512, T*BL)."""
    tw = min(512, t_steps * BL)
    assert (t_steps * BL) % tw == 0

    nc = bacc.Bacc("TRN2", target_bir_lowering=False, debug=False)

    xT = nc.dram_tensor("xT", [KC, 128, t_steps, BL], f32r, kind="ExternalInput")
    winT = nc.dram_tensor("winT", [KC, 128, H], f32r, kind="ExternalInput")
    wrecT = nc.dram_tensor("wrecT", [HC, 128, H], f16, kind="ExternalInput")
    uzT = nc.dram_tensor("uzT", [HC, 128, H], f16, kind="ExternalInput")
    bin2 = nc.dram_tensor("bin2", [HC, 128], f32, kind="ExternalInput")
    bu2 = nc.dram_tensor("bu2", [HC, 128], f32, kind="ExternalInput")
    ident = nc.dram_tensor("ident", [128, 128], f16, kind="ExternalInput")
    zout = nc.dram_tensor("zoutT", [HC, 128, t_steps, BL], f16,
                          kind="ExternalOutput")
    mout = nc.dram_tensor("moutT", [HC, 128, t_steps, BL], f16,
                          kind="ExternalOutput")

    with tile.TileContext(nc) as tc:
        _body(tc, nc, xT, winT, wrecT, uzT, bin2, bu2, ident, zout, mout,
              t_steps, tw, sweeps)

    if compile:
        nc.compile()
    return nc


def _body(tc, nc, xT, winT, wrecT, uzT, bin2, bu2, ident, zout, mout,
          t_steps, tw, sweeps):
    from contextlib import ExitStack

    nt = (t_steps * BL) // tw          # number of t-tiles
    tws = tw // BL                     # steps per tile
    tp1 = t_steps + 1                  # padded length of m in t
    n_half = max(1, nt // 2)           # tiles per scan half
    th = n_half * tws                  # steps per scan half

    with ExitStack() as ctx:
        cpool = ctx.enter_context(tc.tile_pool(name="consts", bufs=1))
        mpool = ctx.enter_context(tc.tile_pool(name="master", bufs=1))
        xpool = ctx.enter_context(tc.tile_pool(name="xin", bufs=2))
        zpool = ctx.enter_context(tc.tile_pool(name="ztile", bufs=3))
        pqpool = ctx.enter_context(tc.tile_pool(name="psum", bufs=2, space="PSUM"))

        # ---- constants in SBUF ----
        # Only w_in + binS gate the first GEMM; the rest is DMA'd after the
        # first tile's emission so phase A starts as early as possible.
        w_in = cpool.tile([128, KC * H], f32r, tag="w_in")
        w_rec = cpool.tile([128, HC * H], f16, tag="w_rec")
        u_z = cpool.tile([128, HC * H], f16, tag="u_z")
        for k in range(KC):
            nc.sync.dma_start(w_in[:, k * H:(k + 1) * H], winT[k])
        binS = cpool.tile([128, HC], f32, tag="binS")
        nc.sync.dma_start(binS[:], bin2.ap().rearrange("c p -> p c"))
        buS = cpool.tile([128, HC], f32, tag="buS")
        idn = cpool.tile([128, 128], f16, tag="idn")

        def late_const_dmas():
            for k in range(KC):
                nc.sync.dma_start(w_rec[:, k * H:(k + 1) * H], wrecT[k])
                nc.sync.dma_start(u_z[:, k * H:(k + 1) * H], uzT[k])
            nc.sync.dma_start(buS[:], bu2.ap().rearrange("c p -> p c"))
            nc.sync.dma_start(idn[:], ident[:])

        # ---- SBUF masters ----
        # atil/zd/u: [128, (c, t, b)];  m ping/pong: [128, (c, 1+t, b)]
        atil = mpool.tile([128, HC * t_steps * BL], f16, tag="atil")
        zd = mpool.tile([128, HC * t_steps * BL], f16, tag="zd")
        um = mpool.tile([128, HC * t_steps * BL], f16, tag="um")
        m_a = mpool.tile([128, HC * tp1 * BL], f16, tag="m_a")
        m_b = mpool.tile([128, HC * tp1 * BL], f16, tag="m_b")
        a3 = atil[:].rearrange("p (c t b) -> p c t b", c=HC, b=BL)
        zd3 = zd[:].rearrange("p (c t b) -> p c t b", c=HC, b=BL)
        u3 = um[:].rearrange("p (c t b) -> p c t b", c=HC, b=BL)
        ma3 = m_a[:].rearrange("p (c t b) -> p c t b", c=HC, b=BL)
        mb3 = m_b[:].rearrange("p (c t b) -> p c t b", c=HC, b=BL)
        # zero the t=-1 pad columns
        nc.vector.memset(ma3[:, :, 0, :], 0.0)
        nc.vector.memset(mb3[:, :, 0, :], 0.0)

        xr = xT.ap().rearrange("k p t b -> p k t b")
        zo = zout.ap()
        mo = mout.ap()

        # ================= phase A: z, atil, zd =================
        # z16 tiles carry a leading pad column holding the previous tile's
        # last step, so the shifted W_rec matmul and the zd sub need no
        # boundary split.  Emission is skewed one tile: GEMM2(i-1) is emitted
        # after GEMM1(i) so the PE never waits on the tanh of the tile it is
        # about to consume.
        ztiles = []

        def phase_a_front(ti):
            t0 = ti * tws
            xs = xpool.tile([128, KC * tw], f32r, tag="xs")
            nc.sync.dma_start(
                xs[:].rearrange("p (k f) -> p k f", k=KC),
                xr[:, :, t0:t0 + tws, :],
            )
            psA = pqpool.tile([128, HC, tw], f32, tag="ps")
            for cm in range(HC):
                for k in range(KC):
                    nc.tensor.matmul(
                        psA[:, cm, :],
                        w_in[:, k * H + cm * 128:k * H + cm * 128 + 128],
                        xs[:, k * tw:(k + 1) * tw],
                        start=(k == 0),
                        stop=(k == KC - 1),
                        skip_group_check=True,
                    )
            z16 = zpool.tile([128, HC * (tw + BL)], f16, tag="z16")
            zp = z16[:].rearrange("p (c t b) -> p c t b", c=HC, b=BL)
            for cm in range(HC):
                nc.scalar.activation(
                    zp[:, cm, 1:, :].rearrange("p t b -> p (t b)"),
                    psA[:, cm, :], AF.Tanh,
                    bias=binS[:, cm:cm + 1], scale=1.0)
            # leading pad column: previous tile's last step (zeros at t=0)
            if ti == 0:
                nc.vector.memset(zp[:, :, 0, :], 0.0)
            else:
                nc.vector.tensor_copy(zp[:, :, 0, :], ztiles[ti - 1][:, :, tws, :])
            nc.sync.dma_start(zo[:, :, t0:t0 + tws, :]
                              .rearrange("c p t b -> p c t b"),
                              zp[:, :, 1:, :])
            # zd = z_{t-1} - z_t, one op over the padded tile
            nc.vector.tensor_sub(zd3[:, :, t0:t0 + tws, :],
                                 zp[:, :, 0:tws, :],
                                 zp[:, :, 1:tws + 1, :])
            ztiles.append(zp)

        def phase_a_back(ti):
            t0 = ti * tws
            zp = ztiles[ti]
            psB = pqpool.tile([128, HC, tw], f32, tag="ps")
            for cm in range(HC):
                for k in range(HC):
                    nc.tensor.matmul(
                        psB[:, cm, :],
                        u_z[:, k * H + cm * 128:k * H + cm * 128 + 128],
                        zp[:, k, 1:, :].rearrange("p t b -> p (t b)"),
                        start=(k == 0),
                        stop=False,
                        skip_group_check=True,
                    )
                for k in range(HC):
                    nc.tensor.matmul(
                        psB[:, cm, :],
                        w_rec[:, k * H + cm * 128:k * H + cm * 128 + 128],
                        zp[:, k, 0:tws, :].rearrange("p t b -> p (t b)"),
                        start=False, stop=(k == HC - 1),
                        skip_group_check=True,
                    )
                nc.scalar.activation(
                    a3[:, cm, t0:t0 + tws, :].rearrange("p t b -> p (t b)"),
                    psB[:, cm, :], AF.Identity,
                    bias=buS[:, cm:cm + 1], scale=1.0)

        # ================= sweeps =================
        def gemm_tile(m_in3, ti):
            """u[tile] = sigmoid(atil + W_rec m_in_shift) for one t-tile."""
            t0 = ti * tws
            ps = pqpool.tile([128, HC, tw], f32, tag="ps")
            # each cm group: seed PSUM with atil (identity matmul), accumulate
            for cm in range(HC):
                nc.tensor.matmul(
                    ps[:, cm, :], idn[:],
                    a3[:, cm, t0:t0 + tws, :].rearrange("p t b -> p (t b)"),
                    start=True, stop=False, skip_group_check=True)
                for k in range(HC):
                    nc.tensor.matmul(
                        ps[:, cm, :],
                        w_rec[:, k * H + cm * 128:k * H + cm * 128 + 128],
                        m_in3[:, k, t0:t0 + tws, :].rearrange("p t b -> p (t b)"),
                        start=False,
                        stop=(k == HC - 1),
                        skip_group_check=True,
                    )
            nc.scalar.activation(
                u3[:, :, t0:t0 + tws, :],
                ps[:, :, :].rearrange("p c (t b) -> p c t b", b=BL),
                AF.Sigmoid)

        def scan_seg(m_out3, s0, s1):
            """Exact diagonal solve m_t = (zd_t + m_{t-1}) * u_t over steps
            [s0, s1), one DVE scan per (c, b) series."""
            if s0 >= s1:
                return
            for c in range(HC):
                for b in range(BL):
                    init = 0.0 if s0 == 0 else m_out3[:, c, s0:s0 + 1, b]
                    nc.vector.tensor_tensor_scan(
                        m_out3[:, c, 1 + s0:1 + s1, b],
                        zd3[:, c, s0:s1, b],
                        u3[:, c, s0:s1, b],
                        init,
                        op0=aot.AluOpType.add,
                        op1=aot.AluOpType.mult)

        def scan_half(m_out3, h):
            s0 = h * th
            scan_seg(m_out3, s0, min(t_steps, s0 + th))


        def last_sweep_tiled(m_in3, m_out3):
            """Final sweep with per-tile chained scans: the scan chain starts
            after ACT(tile 0) instead of ACT(tile 3), shrinking the tail."""
            for ti in range(nt):
                gemm_tile(m_in3, ti)
                s0, s1 = ti * tws, (ti + 1) * tws
                scan_seg(m_out3, s0, s1)
                nc.sync.dma_start(mo[:, :, s0:s1, :]
                                  .rearrange("c p t b -> p c t b"),
                                  m_out3[:, :, 1 + s0:1 + s1, :])

        # phase A with the GEMM-free sweep 1 folded in: as soon as a tile's
        # atil lands, its sigmoid runs on the ACT (which has slack during
        # phase A), and the scan halves run on the otherwise-idle DVE.
        def presweep_sig(ti):
            t0 = ti * tws
            nc.scalar.activation(u3[:, :, t0:t0 + tws, :],
                                 a3[:, :, t0:t0 + tws, :], AF.Sigmoid)

        for ti in range(nt):
            phase_a_front(ti)
            if ti == 0:
                late_const_dmas()
            if ti > 0:
                phase_a_back(ti - 1)
                presweep_sig(ti - 1)
            if ti - 1 == n_half - 1:
                scan_half(ma3, 0)
        phase_a_back(nt - 1)
        presweep_sig(nt - 1)
        if n_half - 1 == nt - 1:
            scan_half(ma3, 0)
        scan_half(ma3, 1)

        # GEMM sweeps, ping-ponging m_a/m_b
        mm = [ma3, mb3]
        n_gemm = sweeps - 1
        for j in range(n_gemm):
            m_in3, m_out3 = mm[j % 2], mm[(j + 1) % 2]
            last = j == n_gemm - 1
            if last and nt > 1:
                last_sweep_tiled(m_in3, m_out3)
                break
            for ti in range(n_half):
                gemm_tile(m_in3, ti)
            scan_half(m_out3, 0)
            if last:
                nc.sync.dma_start(mo[:, :, 0:th, :]
                                  .rearrange("c p t b -> p c t b"),
                                  m_out3[:, :, 1:1 + th, :])
            if last and th < t_steps:
                # per-tile scan segments + interleaved output DMAs so the
                # final half's scan chain is not a serial tail
                for ti in range(n_half, nt):
                    gemm_tile(m_in3, ti)
                    s0, s1 = ti * tws, (ti + 1) * tws
                    scan_seg(m_out3, s0, s1)
                    nc.sync.dma_start(mo[:, :, s0:s1, :]
                                      .rearrange("c p t b -> p c t b"),
                                      m_out3[:, :, 1 + s0:1 + s1, :])
            else:
                for ti in range(n_half, nt):
                    gemm_tile(m_in3, ti)
                scan_half(m_out3, 1)



_CACHED = {}


def _get_nc(t_steps=T, sweeps=SWEEPS):
    key = (t_steps, sweeps)
    if key not in _CACHED:
        _CACHED[key] = build(t_steps, sweeps=sweeps)
    return _CACHED[key]


def make_in_maps(inputs, W_in, b_in, W_rec, U_z, b_u, t_steps=T):
    x = np.asarray(inputs, dtype=np.float32)
    winT_np = np.ascontiguousarray(
        np.asarray(W_in, np.float32).T.reshape(KC, 128, H))
    wrecT_np = np.ascontiguousarray(
        np.asarray(W_rec, np.float32).T.reshape(HC, 128, H)).astype(np.float16)
    uzT_np = np.ascontiguousarray(
        np.asarray(U_z, np.float32).T.reshape(HC, 128, H)).astype(np.float16)
    bin_np = np.ascontiguousarray(np.asarray(b_in, np.float32).reshape(HC, 128))
    bu_np = np.ascontiguousarray(np.asarray(b_u, np.float32).reshape(HC, 128))
    id_np = np.eye(128, dtype=np.float32).astype(np.float16)

    in_maps = []
    for c in range(N_CORES):
        xc = x[c * BL:(c + 1) * BL, :t_steps, :]          # (BL, t, I)
        xTc = np.ascontiguousarray(xc.transpose(2, 1, 0)  # (I, t, BL)
                                   ).reshape(KC, 128, t_steps, BL)
        in_maps.append({
            "xT": xTc, "winT": winT_np, "wrecT": wrecT_np, "uzT": uzT_np,
            "bin2": bin_np, "bu2": bu_np, "ident": id_np,
        })
    return in_maps


def gather_out(res, n_cores=N_CORES):
    """Host-side s = z + m in fp32 from the two fp16 device outputs."""
    outs = []
    for c in range(n_cores):
        zc = _unpack(res.results[c]["zoutT"])
        mc = _unpack(res.results[c]["moutT"])
        outs.append(zc + mc)
    return np.ascontiguousarray(np.concatenate(outs, axis=0), dtype=np.float32)


def _unpack(oT):
    # [HC, 128, t, BL] -> [BL, t, HC*128] fp32
    hc, p, t, bl = oT.shape
    return oT.transpose(3, 2, 0, 1).reshape(bl, t, hc * p).astype(np.float32)


def kernel(inputs, W_in, b_in, W_rec, U_z, b_u):
    nc = _get_nc()
    in_maps = make_in_maps(inputs, W_in, b_in, W_rec, U_z, b_u)
    res = bass_utils.run_bass_kernel_spmd(nc, in_maps, core_ids=list(range(N_CORES)))
    return gather_out(res)


# revision 32
# speedup vs baseline: 1.0044x; 1.0044x over previous
"""MinRNN Trainium2 kernel — parallel-in-time Picard iteration with exact
per-sweep diagonal solves on the DVE hardware scan.

Model (per batch row b):
    z_t = tanh(x_t @ W_in^T + b_in)                      # no recurrence
    u_t = sigmoid(s_{t-1} @ W_rec^T + z_t @ U_z^T + b_u)
    s_t = u_t * s_{t-1} + (1 - u_t) * z_t

Reformulate with m_t := s_t - z_t  (so s_t = z_t + m_t):
    pre_t = atil_t + W_rec m_{t-1},   atil_t = W_rec z_{t-1} + U_z z_t + b_u
    m_t   = sigmoid(pre_t) * (zd_t + m_{t-1}),  zd_t = z_{t-1} - z_t
(z_{-1} = 0, m_{-1} = 0 gives s_{-1} = 0.)

A sequential scan is LDWEIGHTS-bound on the PE (~850ns/step streaming all of
W_rec against a 2-wide operand => ~1.75ms floor).  Instead iterate on the
whole trajectory: per sweep,
    u^{(j)}_t = sigmoid(atil_t + W_rec m^{(j-1)}_{t-1})      # dense GEMM, 512-wide tiles
    m^{(j)}_t = (zd_t + m^{(j)}_{t-1}) * u^{(j)}_t           # EXACT diagonal solve:
                                                             # DVE tensor_tensor_scan
The exact diagonal solve makes the outer iteration contract ~5x per sweep on
this data (vs ~1.5x for plain Jacobi): 5 GEMM sweeps reach ~2.3e-3 max-rel
(gate is 2e-2).  atil is injected into PSUM with identity matmuls so the only
DVE work per sweep is the scan itself; scans run in two T/2 halves so they
overlap the next sweep's GEMM (its tiles 0-3 need only half 1, since tile i
reads m over t in [256*i-1, 256*(i+1)-2]).  All sweep tensors are fp16 (bf16
noise plateaus at the 2e-2 gate; fp16 at ~2e-3; scan state is fp32 in hw).

The kernel outputs z (fp16, written during phase A) and the final m (fp16);
the host computes s = z + m in fp32 — this removes a serialized on-device
output pass and one fp16 rounding.

Sharding: data-parallel over batch, 2 rows per core, no collectives.
"""

import numpy as np

import concourse.bass as bass
import concourse.mybir as mybir
import concourse.tile as tile
import concourse.bacc as bacc
from concourse import bass_utils
import concourse.alu_op_type as aot

AF = mybir.ActivationFunctionType
ET = mybir.EngineType

B, T, I, H = 16, 2048, 512, 512
N_CORES = 8
BL = B // N_CORES          # batch rows per core (2)
KC = I // 128              # input-dim chunks (4)
HC = H // 128              # hidden-dim chunks (4)
SWEEPS = 4                 # total sweeps incl. the GEMM-free first one

f32 = mybir.dt.float32
f32r = mybir.dt.float32r
f16 = mybir.dt.float16


def build(t_steps: int = T, tb: int = 0, sweeps: int = SWEEPS, compile: bool = True):
    """Build the per-core Bass program (same program on all 8 cores).
    `tb` accepted for CLI compat; tile width is fixed at min(512, T*BL)."""
    tw = min(512, t_steps * BL)
    assert (t_steps * BL) % tw == 0

    nc = bacc.Bacc("TRN2", target_bir_lowering=False, debug=False)

    xT = nc.dram_tensor("xT", [KC, 128, t_steps, BL], f32r, kind="ExternalInput")
    winT = nc.dram_tensor("winT", [KC, 128, H], f32r, kind="ExternalInput")
    wrecT = nc.dram_tensor("wrecT", [HC, 128, H], f16, kind="ExternalInput")
    uzT = nc.dram_tensor("uzT", [HC, 128, H], f16, kind="ExternalInput")
    bin2 = nc.dram_tensor("bin2", [HC, 128], f32, kind="ExternalInput")
    bu2 = nc.dram_tensor("bu2", [HC, 128], f32, kind="ExternalInput")
    ident = nc.dram_tensor("ident", [128, 128], f16, kind="ExternalInput")
    zout = nc.dram_tensor("zoutT", [HC, 128, t_steps, BL], f16,
                          kind="ExternalOutput")
    mout = nc.dram_tensor("moutT", [HC, 128, t_steps, BL], f16,
                          kind="ExternalOutput")

    with tile.TileContext(nc) as tc:
        _body(tc, nc, xT, winT, wrecT, uzT, bin2, bu2, ident, zout, mout,
              t_steps, tw, sweeps)

    if compile:
        nc.compile()
    return nc


def _body(tc, nc, xT, winT, wrecT, uzT, bin2, bu2, ident, zout, mout,
          t_steps, tw, sweeps):
    from contextlib import ExitStack

    nt = (t_steps * BL) // tw          # number of t-tiles
    tws = tw // BL                     # steps per tile
    tp1 = t_steps + 1                  # padded length of m in t
    n_half = max(1, nt // 2)           # tiles per scan half
    th = n_half * tws                  # steps per scan half

    with ExitStack() as ctx:
        cpool = ctx.enter_context(tc.tile_pool(name="consts", bufs=1))
        mpool = ctx.enter_context(tc.tile_pool(name="master", bufs=1))
        xpool = ctx.enter_context(tc.tile_pool(name="xin", bufs=2))
        zpool = ctx.enter_context(tc.tile_pool(name="ztile", bufs=3))
        pqpool = ctx.enter_context(tc.tile_pool(name="psum", bufs=2, space="PSUM"))

        # ---- constants in SBUF ----
        # Only w_in + binS gate the first GEMM; the rest is DMA'd after the
        # first tile's emission so phase A starts as early as possible.
        w_in = cpool.tile([128, KC * H], f32r, tag="w_in")
        w_rec = cpool.tile([128, HC * H], f16, tag="w_rec")
        u_z = cpool.tile([128, HC * H], f16, tag="u_z")
        for k in range(KC):
            nc.sync.dma_start(w_in[:, k * H:(k + 1) * H], winT[k])
        binS = cpool.tile([128, HC], f32, tag="binS")
        nc.sync.dma_start(binS[:], bin2.ap().rearrange("c p -> p c"))
        buS = cpool.tile([128, HC], f32, tag="buS")
        idn = cpool.tile([128, 128], f16, tag="idn")

        def late_const_dmas():
            for k in range(KC):
                nc.sync.dma_start(w_rec[:, k * H:(k + 1) * H], wrecT[k])
                nc.sync.dma_start(u_z[:, k * H:(k + 1) * H], uzT[k])
            nc.sync.dma_start(buS[:], bu2.ap().rearrange("c p -> p c"))
            nc.sync.dma_start(idn[:], ident[:])

        # ---- SBUF masters ----
        # atil/zd/u: [128, (c, t, b)];  m ping/pong: [128, (c, 1+t, b)]
        atil = mpool.tile([128, HC * t_steps * BL], f16, tag="atil")
        zd = mpool.tile([128, HC * t_steps * BL], f16, tag="zd")
        um = mpool.tile([128, HC * t_steps * BL], f16, tag="um")
        m_a = mpool.tile([128, HC * tp1 * BL], f16, tag="m_a")
        m_b = mpool.tile([128, HC * tp1 * BL], f16, tag="m_b")
        a3 = atil[:].rearrange("p (c t b) -> p c t b", c=HC, b=BL)
        zd3 = zd[:].rearrange("p (c t b) -> p c t b", c=HC, b=BL)
        u3 = um[:].rearrange("p (c t b) -> p c t b", c=HC, b=BL)
        ma3 = m_a[:].rearrange("p (c t b) -> p c t b", c=HC, b=BL)
        mb3 = m_b[:].rearrange("p (c t b) -> p c t b", c=HC, b=BL)
        # zero the t=-1 pad columns
        nc.vector.memset(ma3[:, :, 0, :], 0.0)
        nc.vector.memset(mb3[:, :, 0, :], 0.0)

        xr = xT.ap().rearrange("k p t b -> p k t b")
        zo = zout.ap()
        mo = mout.ap()

        # ================= phase A: z, atil, zd =================
        # z16 tiles carry a leading pad column holding the previous tile's
        # last step, so the shifted W_rec matmul and the zd sub need no
        # boundary split.  Emission is skewed one tile: GEMM2(i-1) is emitted
        # after GEMM1(i) so the PE never waits on the tanh of the tile it is
        # about to consume.
        ztiles = []

        def phase_a_front(ti):
            t0 = ti * tws
            xs = xpool.tile([128, KC * tw], f32r, tag="xs")
            nc.sync.dma_start(
                xs[:].rearrange("p (k f) -> p k f", k=KC),
                xr[:, :, t0:t0 + tws, :],
            )
            psA = pqpool.tile([128, HC, tw], f32, tag="ps")
            for cm in range(HC):
                for k in range(KC):
                    nc.tensor.matmul(
                        psA[:, cm, :],
                        w_in[:, k * H + cm * 128:k * H + cm * 128 + 128],
                        xs[:, k * tw:(k + 1) * tw],
                        start=(k == 0),
                        stop=(k == KC - 1),
                        skip_group_check=True,
                    )
            z16 = zpool.tile([128, HC * (tw + BL)], f16, tag="z16")
            zp = z16[:].rearrange("p (c t b) -> p c t b", c=HC, b=BL)
            for cm in range(HC):
                nc.scalar.activation(
                    zp[:, cm, 1:, :].rearrange("p t b -> p (t b)"),
                    psA[:, cm, :], AF.Tanh,
                    bias=binS[:, cm:cm + 1], scale=1.0)
            # leading pad column: previous tile's last step (zeros at t=0).
            # On Pool, not DVE: these must not queue behind the pre-sweep
            # scans, or GEMM2 stalls on the pad dependency.
            if ti == 0:
                nc.gpsimd.memset(zp[:, :, 0, :], 0.0)
            else:
                nc.gpsimd.tensor_copy(zp[:, :, 0, :], ztiles[ti - 1][:, :, tws, :])
            nc.sync.dma_start(zo[:, :, t0:t0 + tws, :]
                              .rearrange("c p t b -> p c t b"),
                              zp[:, :, 1:, :])
            # zd = z_{t-1} - z_t, one op over the padded tile (Pool, same
            # reason as the pad copy)
            nc.gpsimd.tensor_sub(zd3[:, :, t0:t0 + tws, :],
                                 zp[:, :, 0:tws, :],
                                 zp[:, :, 1:tws + 1, :])
            ztiles.append(zp)

        def phase_a_back(ti):
            t0 = ti * tws
            zp = ztiles[ti]
            psB = pqpool.tile([128, HC, tw], f32, tag="ps")
            for cm in range(HC):
                for k in range(HC):
                    nc.tensor.matmul(
                        psB[:, cm, :],
                        u_z[:, k * H + cm * 128:k * H + cm * 128 + 128],
                        zp[:, k, 1:, :].rearrange("p t b -> p (t b)"),
                        start=(k == 0),
                        stop=False,
                        skip_group_check=True,
                    )
                for k in range(HC):
                    nc.tensor.matmul(
                        psB[:, cm, :],
                        w_rec[:, k * H + cm * 128:k * H + cm * 128 + 128],
                        zp[:, k, 0:tws, :].rearrange("p t b -> p (t b)"),
                        start=False, stop=(k == HC - 1),
                        skip_group_check=True,
                    )
                nc.scalar.activation(
                    a3[:, cm, t0:t0 + tws, :].rearrange("p t b -> p (t b)"),
                    psB[:, cm, :], AF.Identity,
                    bias=buS[:, cm:cm + 1], scale=1.0)

        # ================= sweeps =================
        def gemm_tile(m_in3, ti):
            """u[tile] = sigmoid(atil + W_rec m_in_shift) for one t-tile."""
            t0 = ti * tws
            ps = pqpool.tile([128, HC, tw], f32, tag="ps")
            # each cm group: seed PSUM with atil (identity matmul), accumulate
            for cm in range(HC):
                nc.tensor.matmul(
                    ps[:, cm, :], idn[:],
                    a3[:, cm, t0:t0 + tws, :].rearrange("p t b -> p (t b)"),
                    start=True, stop=False, skip_group_check=True)
                for k in range(HC):
                    nc.tensor.matmul(
                        ps[:, cm, :],
                        w_rec[:, k * H + cm * 128:k * H + cm * 128 + 128],
                        m_in3[:, k, t0:t0 + tws, :].rearrange("p t b -> p (t b)"),
                        start=False,
                        stop=(k == HC - 1),
                        skip_group_check=True,
                    )
            nc.scalar.activation(
                u3[:, :, t0:t0 + tws, :],
                ps[:, :, :].rearrange("p c (t b) -> p c t b", b=BL),
                AF.Sigmoid)

        def scan_seg(m_out3, s0, s1):
            """Exact diagonal solve m_t = (zd_t + m_{t-1}) * u_t over steps
            [s0, s1), one DVE scan per (c, b) series."""
            if s0 >= s1:
                return
            for c in range(HC):
                for b in range(BL):
                    init = 0.0 if s0 == 0 else m_out3[:, c, s0:s0 + 1, b]
                    nc.vector.tensor_tensor_scan(
                        m_out3[:, c, 1 + s0:1 + s1, b],
                        zd3[:, c, s0:s1, b],
                        u3[:, c, s0:s1, b],
                        init,
                        op0=aot.AluOpType.add,
                        op1=aot.AluOpType.mult)

        def scan_half(m_out3, h):
            s0 = h * th
            scan_seg(m_out3, s0, min(t_steps, s0 + th))


        def last_sweep_tiled(m_in3, m_out3):
            """Final sweep with per-tile chained scans: the scan chain starts
            after ACT(tile 0) instead of ACT(tile 3), shrinking the tail."""
            for ti in range(nt):
                gemm_tile(m_in3, ti)
                s0, s1 = ti * tws, (ti + 1) * tws
                scan_seg(m_out3, s0, s1)
                nc.sync.dma_start(mo[:, :, s0:s1, :]
                                  .rearrange("c p t b -> p c t b"),
                                  m_out3[:, :, 1 + s0:1 + s1, :])

        # phase A with the GEMM-free sweep 1 folded in: as soon as a tile's
        # atil lands, its sigmoid runs on the ACT (which has slack during
        # phase A), and the scan halves run on the otherwise-idle DVE.
        def presweep_sig(ti):
            t0 = ti * tws
            nc.scalar.activation(u3[:, :, t0:t0 + tws, :],
                                 a3[:, :, t0:t0 + tws, :], AF.Sigmoid)

        for ti in range(nt):
            phase_a_front(ti)
            if ti == 0:
                late_const_dmas()
            if ti > 0:
                phase_a_back(ti - 1)
                presweep_sig(ti - 1)
            if ti - 1 == n_half - 1:
                scan_half(ma3, 0)
        phase_a_back(nt - 1)
        presweep_sig(nt - 1)
        if n_half - 1 == nt - 1:
            scan_half(ma3, 0)
        scan_half(ma3, 1)

        # GEMM sweeps, ping-ponging m_a/m_b
        mm = [ma3, mb3]
        n_gemm = sweeps - 1
        for j in range(n_gemm):
            m_in3, m_out3 = mm[j % 2], mm[(j + 1) % 2]
            last = j == n_gemm - 1
            if last and nt > 1:
                last_sweep_tiled(m_in3, m_out3)
                break
            for ti in range(n_half):
                gemm_tile(m_in3, ti)
            scan_half(m_out3, 0)
            if last:
                nc.sync.dma_start(mo[:, :, 0:th, :]
                                  .rearrange("c p t b -> p c t b"),
                                  m_out3[:, :, 1:1 + th, :])
            if last and th < t_steps:
                # per-tile scan segments + interleaved output DMAs so the
                # final half's scan chain is not a serial tail
                for ti in range(n_half, nt):
                    gemm_tile(m_in3, ti)
                    s0, s1 = ti * tws, (ti + 1) * tws
                    scan_seg(m_out3, s0, s1)
                    nc.sync.dma_start(mo[:, :, s0:s1, :]
                                      .rearrange("c p t b -> p c t b"),
                                      m_out3[:, :, 1 + s0:1 + s1, :])
            else:
                for ti in range(n_half, nt):
                    gemm_tile(m_in3, ti)
                scan_half(m_out3, 1)



_CACHED = {}


def _get_nc(t_steps=T, sweeps=SWEEPS):
    key = (t_steps, sweeps)
    if key not in _CACHED:
        _CACHED[key] = build(t_steps, sweeps=sweeps)
    return _CACHED[key]


def make_in_maps(inputs, W_in, b_in, W_rec, U_z, b_u, t_steps=T):
    x = np.asarray(inputs, dtype=np.float32)
    winT_np = np.ascontiguousarray(
        np.asarray(W_in, np.float32).T.reshape(KC, 128, H))
    wrecT_np = np.ascontiguousarray(
        np.asarray(W_rec, np.float32).T.reshape(HC, 128, H)).astype(np.float16)
    uzT_np = np.ascontiguousarray(
        np.asarray(U_z, np.float32).T.reshape(HC, 128, H)).astype(np.float16)
    bin_np = np.ascontiguousarray(np.asarray(b_in, np.float32).reshape(HC, 128))
    bu_np = np.ascontiguousarray(np.asarray(b_u, np.float32).reshape(HC, 128))
    id_np = np.eye(128, dtype=np.float32).astype(np.float16)

    in_maps = []
    for c in range(N_CORES):
        xc = x[c * BL:(c + 1) * BL, :t_steps, :]          # (BL, t, I)
        xTc = np.ascontiguousarray(xc.transpose(2, 1, 0)  # (I, t, BL)
                                   ).reshape(KC, 128, t_steps, BL)
        in_maps.append({
            "xT": xTc, "winT": winT_np, "wrecT": wrecT_np, "uzT": uzT_np,
            "bin2": bin_np, "bu2": bu_np, "ident": id_np,
        })
    return in_maps


def gather_out(res, n_cores=N_CORES):
    """Host-side s = z + m in fp32 from the two fp16 device outputs."""
    outs = []
    for c in range(n_cores):
        zc = _unpack(res.results[c]["zoutT"])
        mc = _unpack(res.results[c]["moutT"])
        outs.append(zc + mc)
    return np.ascontiguousarray(np.concatenate(outs, axis=0), dtype=np.float32)


def _unpack(oT):
    # [HC, 128, t, BL] -> [BL, t, HC*128] fp32
    hc, p, t, bl = oT.shape
    return oT.transpose(3, 2, 0, 1).reshape(bl, t, hc * p).astype(np.float32)


def kernel(inputs, W_in, b_in, W_rec, U_z, b_u):
    nc = _get_nc()
    in_maps = make_in_maps(inputs, W_in, b_in, W_rec, U_z, b_u)
    res = bass_utils.run_bass_kernel_spmd(nc, in_maps, core_ids=list(range(N_CORES)))
    return gather_out(res)


# revision 35
# speedup vs baseline: 1.1089x; 1.1040x over previous
"""MinRNN Trainium2 kernel — parallel-in-time Picard iteration with exact
per-sweep diagonal solves on the DVE hardware scan.

Model (per batch row b):
    z_t = tanh(x_t @ W_in^T + b_in)                      # no recurrence
    u_t = sigmoid(s_{t-1} @ W_rec^T + z_t @ U_z^T + b_u)
    s_t = u_t * s_{t-1} + (1 - u_t) * z_t

Reformulate with m_t := s_t - z_t  (so s_t = z_t + m_t):
    pre_t = atil_t + W_rec m_{t-1},   atil_t = W_rec z_{t-1} + U_z z_t + b_u
    m_t   = sigmoid(pre_t) * (zd_t + m_{t-1}),  zd_t = z_{t-1} - z_t
(z_{-1} = 0, m_{-1} = 0 gives s_{-1} = 0.)

A sequential scan is LDWEIGHTS-bound on the PE (~850ns/step streaming all of
W_rec against a 2-wide operand => ~1.75ms floor).  Instead iterate on the
whole trajectory: per sweep,
    u^{(j)}_t = sigmoid(atil_t + W_rec m^{(j-1)}_{t-1})      # dense GEMM, 512-wide tiles
    m^{(j)}_t = (zd_t + m^{(j)}_{t-1}) * u^{(j)}_t           # EXACT diagonal solve:
                                                             # DVE tensor_tensor_scan
The exact diagonal solve makes the outer iteration contract ~5x per sweep on
this data (vs ~1.5x for plain Jacobi): 5 GEMM sweeps reach ~2.3e-3 max-rel
(gate is 2e-2).  atil is injected into PSUM with identity matmuls so the only
DVE work per sweep is the scan itself; scans run in two T/2 halves so they
overlap the next sweep's GEMM (its tiles 0-3 need only half 1, since tile i
reads m over t in [256*i-1, 256*(i+1)-2]).  All sweep tensors are fp16 (bf16
noise plateaus at the 2e-2 gate; fp16 at ~2e-3; scan state is fp32 in hw).

The kernel outputs z (fp16, written during phase A) and the final m (fp16);
the host computes s = z + m in fp32 — this removes a serialized on-device
output pass and one fp16 rounding.

Sharding: data-parallel over batch, 2 rows per core, no collectives.
"""

import numpy as np

import concourse.bass as bass
import concourse.mybir as mybir
import concourse.tile as tile
import concourse.bacc as bacc
from concourse import bass_utils
import concourse.alu_op_type as aot

AF = mybir.ActivationFunctionType
ET = mybir.EngineType

B, T, I, H = 16, 2048, 512, 512
N_CORES = 8
BL = B // N_CORES          # batch rows per core (2)
KC = I // 128              # input-dim chunks (4)
HC = H // 128              # hidden-dim chunks (4)
SWEEPS = 4                 # total sweeps incl. the GEMM-free first one

f32 = mybir.dt.float32
f32r = mybir.dt.float32r
f16 = mybir.dt.float16


def build(t_steps: int = T, tb: int = 0, sweeps: int = SWEEPS, compile: bool = True):
    """Build the per-core Bass program (same program on all 8 cores).
    `tb` accepted for CLI compat; tile width is fixed at min(512, T*BL)."""
    tw = min(512, t_steps * BL)
    assert (t_steps * BL) % tw == 0

    nc = bacc.Bacc("TRN2", target_bir_lowering=False, debug=False)

    xT = nc.dram_tensor("xT", [KC, 128, t_steps, BL], f32r, kind="ExternalInput")
    winT = nc.dram_tensor("winT", [KC, 128, H], f32r, kind="ExternalInput")
    wrecT = nc.dram_tensor("wrecT", [HC, 128, H], f16, kind="ExternalInput")
    uzT = nc.dram_tensor("uzT", [HC, 128, H], f16, kind="ExternalInput")
    bin2 = nc.dram_tensor("bin2", [HC, 128], f32, kind="ExternalInput")
    bu2 = nc.dram_tensor("bu2", [HC, 128], f32, kind="ExternalInput")
    ident = nc.dram_tensor("ident", [128, 128], f16, kind="ExternalInput")
    zout = nc.dram_tensor("zoutT", [HC, 128, t_steps, BL], f16,
                          kind="ExternalOutput")
    mout = nc.dram_tensor("moutT", [HC, 128, t_steps, BL], f16,
                          kind="ExternalOutput")

    with tile.TileContext(nc) as tc:
        _body(tc, nc, xT, winT, wrecT, uzT, bin2, bu2, ident, zout, mout,
              t_steps, tw, sweeps)

    if compile:
        nc.compile()
    return nc


def _body(tc, nc, xT, winT, wrecT, uzT, bin2, bu2, ident, zout, mout,
          t_steps, tw, sweeps):
    from contextlib import ExitStack

    nt = (t_steps * BL) // tw          # number of t-tiles
    tws = tw // BL                     # steps per tile
    tp1 = t_steps + 1                  # padded length of m in t
    n_half = max(1, nt // 2)           # tiles per scan half
    th = n_half * tws                  # steps per scan half

    with ExitStack() as ctx:
        cpool = ctx.enter_context(tc.tile_pool(name="consts", bufs=1))
        mpool = ctx.enter_context(tc.tile_pool(name="master", bufs=1))
        xpool = ctx.enter_context(tc.tile_pool(name="xin", bufs=2))
        zpool = ctx.enter_context(tc.tile_pool(name="ztile", bufs=3))
        pqpool = ctx.enter_context(tc.tile_pool(name="psum", bufs=8, space="PSUM"))

        # ---- constants in SBUF ----
        # Only w_in + binS gate the first GEMM; the rest is DMA'd after the
        # first tile's emission so phase A starts as early as possible.
        w_in = cpool.tile([128, KC * H], f32r, tag="w_in")
        w_rec = cpool.tile([128, HC * H], f16, tag="w_rec")
        u_z = cpool.tile([128, HC * H], f16, tag="u_z")
        for k in range(KC):
            nc.sync.dma_start(w_in[:, k * H:(k + 1) * H], winT[k])
        binS = cpool.tile([128, HC], f32, tag="binS")
        nc.sync.dma_start(binS[:], bin2.ap().rearrange("c p -> p c"))
        buS = cpool.tile([128, HC], f32, tag="buS")
        idn = cpool.tile([128, 128], f16, tag="idn")

        def late_const_dmas():
            for k in range(KC):
                nc.sync.dma_start(w_rec[:, k * H:(k + 1) * H], wrecT[k])
                nc.sync.dma_start(u_z[:, k * H:(k + 1) * H], uzT[k])
            nc.sync.dma_start(buS[:], bu2.ap().rearrange("c p -> p c"))
            nc.sync.dma_start(idn[:], ident[:])

        # ---- SBUF masters ----
        # atil/zd/u: [128, (c, t, b)];  m ping/pong: [128, (c, 1+t, b)]
        atil = mpool.tile([128, HC * t_steps * BL], f16, tag="atil")
        zd = mpool.tile([128, HC * t_steps * BL], f16, tag="zd")
        um = mpool.tile([128, HC * t_steps * BL], f16, tag="um")
        m_a = mpool.tile([128, HC * tp1 * BL], f16, tag="m_a")
        m_b = mpool.tile([128, HC * tp1 * BL], f16, tag="m_b")
        a3 = atil[:].rearrange("p (c t b) -> p c t b", c=HC, b=BL)
        zd3 = zd[:].rearrange("p (c t b) -> p c t b", c=HC, b=BL)
        u3 = um[:].rearrange("p (c t b) -> p c t b", c=HC, b=BL)
        ma3 = m_a[:].rearrange("p (c t b) -> p c t b", c=HC, b=BL)
        mb3 = m_b[:].rearrange("p (c t b) -> p c t b", c=HC, b=BL)
        # zero the t=-1 pad columns
        nc.vector.memset(ma3[:, :, 0, :], 0.0)
        nc.vector.memset(mb3[:, :, 0, :], 0.0)

        xr = xT.ap().rearrange("k p t b -> p k t b")
        zo = zout.ap()
        mo = mout.ap()

        # ================= phase A: z, atil, zd =================
        # z16 tiles carry a leading pad column holding the previous tile's
        # last step, so the shifted W_rec matmul and the zd sub need no
        # boundary split.  Emission is skewed one tile: GEMM2(i-1) is emitted
        # after GEMM1(i) so the PE never waits on the tanh of the tile it is
        # about to consume.
        ztiles = []

        def phase_a_front(ti):
            t0 = ti * tws
            xs = xpool.tile([128, KC * tw], f32r, tag="xs")
            nc.sync.dma_start(
                xs[:].rearrange("p (k f) -> p k f", k=KC),
                xr[:, :, t0:t0 + tws, :],
            )
            z16 = zpool.tile([128, HC * (tw + BL)], f16, tag="z16")
            zp = z16[:].rearrange("p (c t b) -> p c t b", c=HC, b=BL)
            for cm in range(HC):
                psA = pqpool.tile([128, tw], f32, tag="ps")
                for k in range(KC):
                    nc.tensor.matmul(
                        psA[:],
                        w_in[:, k * H + cm * 128:k * H + cm * 128 + 128],
                        xs[:, k * tw:(k + 1) * tw],
                        start=(k == 0),
                        stop=(k == KC - 1),
                        skip_group_check=True,
                    )
                nc.scalar.activation(
                    zp[:, cm, 1:, :].rearrange("p t b -> p (t b)"),
                    psA[:], AF.Tanh,
                    bias=binS[:, cm:cm + 1], scale=1.0)
            # leading pad column: previous tile's last step (zeros at t=0)
            if ti == 0:
                nc.vector.memset(zp[:, :, 0, :], 0.0)
            else:
                nc.vector.tensor_copy(zp[:, :, 0, :], ztiles[ti - 1][:, :, tws, :])
            nc.sync.dma_start(zo[:, :, t0:t0 + tws, :]
                              .rearrange("c p t b -> p c t b"),
                              zp[:, :, 1:, :])
            # zd = z_{t-1} - z_t, one op over the padded tile
            nc.vector.tensor_sub(zd3[:, :, t0:t0 + tws, :],
                                 zp[:, :, 0:tws, :],
                                 zp[:, :, 1:tws + 1, :])
            ztiles.append(zp)

        def phase_a_back(ti):
            t0 = ti * tws
            zp = ztiles[ti]
            for cm in range(HC):
                psB = pqpool.tile([128, tw], f32, tag="ps")
                for k in range(HC):
                    nc.tensor.matmul(
                        psB[:],
                        u_z[:, k * H + cm * 128:k * H + cm * 128 + 128],
                        zp[:, k, 1:, :].rearrange("p t b -> p (t b)"),
                        start=(k == 0),
                        stop=False,
                        skip_group_check=True,
                    )
                for k in range(HC):
                    nc.tensor.matmul(
                        psB[:],
                        w_rec[:, k * H + cm * 128:k * H + cm * 128 + 128],
                        zp[:, k, 0:tws, :].rearrange("p t b -> p (t b)"),
                        start=False, stop=(k == HC - 1),
                        skip_group_check=True,
                    )
                nc.scalar.activation(
                    a3[:, cm, t0:t0 + tws, :].rearrange("p t b -> p (t b)"),
                    psB[:], AF.Identity,
                    bias=buS[:, cm:cm + 1], scale=1.0)

        # ================= sweeps =================
        def gemm_cm_pass(m_in3, cm, tlo, thi):
            """u[cm, tiles tlo..thi-1] = sigmoid(atil + W_rec m_in_shift),
            one cm chunk across a tile range (cm-major: unlocks chunk-c
            scans after ~1/4 of the half's PE work)."""
            for ti in range(tlo, thi):
                t0 = ti * tws
                ps = pqpool.tile([128, tw], f32, tag="ps")
                nc.tensor.matmul(
                    ps[:], idn[:],
                    a3[:, cm, t0:t0 + tws, :].rearrange("p t b -> p (t b)"),
                    start=True, stop=False, skip_group_check=True)
                for k in range(HC):
                    nc.tensor.matmul(
                        ps[:],
                        w_rec[:, k * H + cm * 128:k * H + cm * 128 + 128],
                        m_in3[:, k, t0:t0 + tws, :].rearrange("p t b -> p (t b)"),
                        start=False,
                        stop=(k == HC - 1),
                        skip_group_check=True,
                    )
                nc.scalar.activation(
                    u3[:, cm, t0:t0 + tws, :].rearrange("p t b -> p (t b)"),
                    ps[:], AF.Sigmoid)

        def scan_seg(m_out3, s0, s1):
            """Exact diagonal solve m_t = (zd_t + m_{t-1}) * u_t over steps
            [s0, s1), one DVE scan per (c, b) series."""
            if s0 >= s1:
                return
            for c in range(HC):
                for b in range(BL):
                    init = 0.0 if s0 == 0 else m_out3[:, c, s0:s0 + 1, b]
                    nc.vector.tensor_tensor_scan(
                        m_out3[:, c, 1 + s0:1 + s1, b],
                        zd3[:, c, s0:s1, b],
                        u3[:, c, s0:s1, b],
                        init,
                        op0=aot.AluOpType.add,
                        op1=aot.AluOpType.mult)

        def scan_half(m_out3, h):
            s0 = h * th
            scan_seg(m_out3, s0, min(t_steps, s0 + th))


        def last_sweep_tiled(m_in3, m_out3):
            """Final sweep with per-tile chained scans: the scan chain starts
            after ACT(tile 0) instead of ACT(tile 3), shrinking the tail."""
            for ti in range(nt):
                gemm_tile(m_in3, ti)
                s0, s1 = ti * tws, (ti + 1) * tws
                scan_seg(m_out3, s0, s1)
                nc.sync.dma_start(mo[:, :, s0:s1, :]
                                  .rearrange("c p t b -> p c t b"),
                                  m_out3[:, :, 1 + s0:1 + s1, :])

        # phase A with the GEMM-free sweep 1 folded in: as soon as a tile's
        # atil lands, its sigmoid runs on the ACT (which has slack during
        # phase A), and the scan halves run on the otherwise-idle DVE.
        def presweep_sig(ti):
            t0 = ti * tws
            nc.scalar.activation(u3[:, :, t0:t0 + tws, :],
                                 a3[:, :, t0:t0 + tws, :], AF.Sigmoid)

        for ti in range(nt):
            phase_a_front(ti)
            if ti == 0:
                late_const_dmas()
            if ti > 0:
                phase_a_back(ti - 1)
                presweep_sig(ti - 1)
            if ti - 1 == n_half - 1:
                scan_half(ma3, 0)
        phase_a_back(nt - 1)
        presweep_sig(nt - 1)
        if n_half - 1 == nt - 1:
            scan_half(ma3, 0)
        scan_half(ma3, 1)

        # GEMM sweeps, ping-ponging m_a/m_b.  cm-major within each half:
        # chunk c's scans start right after its cm pass, overlapping the
        # remaining cm passes and the next half's GEMM.
        mm = [ma3, mb3]
        n_gemm = sweeps - 1
        for j in range(n_gemm):
            m_in3, m_out3 = mm[j % 2], mm[(j + 1) % 2]
            last = j == n_gemm - 1
            for h in range(2):
                tlo = 0 if h == 0 else n_half
                thi = n_half if h == 0 else nt
                if tlo >= thi:
                    continue
                s0, s1 = tlo * tws, thi * tws
                for cm in range(HC):
                    gemm_cm_pass(m_in3, cm, tlo, thi)
                    for b in range(BL):
                        init = 0.0 if s0 == 0 else m_out3[:, cm, s0:s0 + 1, b]
                        nc.vector.tensor_tensor_scan(
                            m_out3[:, cm, 1 + s0:1 + s1, b],
                            zd3[:, cm, s0:s1, b],
                            u3[:, cm, s0:s1, b],
                            init,
                            op0=aot.AluOpType.add,
                            op1=aot.AluOpType.mult)
                if last:
                    nc.sync.dma_start(mo[:, :, s0:s1, :]
                                      .rearrange("c p t b -> p c t b"),
                                      m_out3[:, :, 1 + s0:1 + s1, :])


_CACHED = {}


def _get_nc(t_steps=T, sweeps=SWEEPS):
    key = (t_steps, sweeps)
    if key not in _CACHED:
        _CACHED[key] = build(t_steps, sweeps=sweeps)
    return _CACHED[key]


def make_in_maps(inputs, W_in, b_in, W_rec, U_z, b_u, t_steps=T):
    x = np.asarray(inputs, dtype=np.float32)
    winT_np = np.ascontiguousarray(
        np.asarray(W_in, np.float32).T.reshape(KC, 128, H))
    wrecT_np = np.ascontiguousarray(
        np.asarray(W_rec, np.float32).T.reshape(HC, 128, H)).astype(np.float16)
    uzT_np = np.ascontiguousarray(
        np.asarray(U_z, np.float32).T.reshape(HC, 128, H)).astype(np.float16)
    bin_np = np.ascontiguousarray(np.asarray(b_in, np.float32).reshape(HC, 128))
    bu_np = np.ascontiguousarray(np.asarray(b_u, np.float32).reshape(HC, 128))
    id_np = np.eye(128, dtype=np.float32).astype(np.float16)

    in_maps = []
    for c in range(N_CORES):
        xc = x[c * BL:(c + 1) * BL, :t_steps, :]          # (BL, t, I)
        xTc = np.ascontiguousarray(xc.transpose(2, 1, 0)  # (I, t, BL)
                                   ).reshape(KC, 128, t_steps, BL)
        in_maps.append({
            "xT": xTc, "winT": winT_np, "wrecT": wrecT_np, "uzT": uzT_np,
            "bin2": bin_np, "bu2": bu_np, "ident": id_np,
        })
    return in_maps


def gather_out(res, n_cores=N_CORES):
    """Host-side s = z + m in fp32 from the two fp16 device outputs."""
    outs = []
    for c in range(n_cores):
        zc = _unpack(res.results[c]["zoutT"])
        mc = _unpack(res.results[c]["moutT"])
        outs.append(zc + mc)
    return np.ascontiguousarray(np.concatenate(outs, axis=0), dtype=np.float32)


def _unpack(oT):
    # [HC, 128, t, BL] -> [BL, t, HC*128] fp32
    hc, p, t, bl = oT.shape
    return oT.transpose(3, 2, 0, 1).reshape(bl, t, hc * p).astype(np.float32)


def kernel(inputs, W_in, b_in, W_rec, U_z, b_u):
    nc = _get_nc()
    in_maps = make_in_maps(inputs, W_in, b_in, W_rec, U_z, b_u)
    res = bass_utils.run_bass_kernel_spmd(nc, in_maps, core_ids=list(range(N_CORES)))
    return gather_out(res)
